# revision 14
# baseline (speedup 1.0000x reference)
"""GCN encoder (2x GCNConv+ReLU, then fused mu/logstd heads) on 8 Trainium2
NeuronCores, Bass/Tile SPMD.

Strategy (node-parallel, feature-major pipeline):
  - Nodes sharded by range: core c owns rows [c*6250, (c+1)*6250), padded to
    6272 = 49 blocks of 128. All on-chip activations are FEATURE-major
    ([feat, node] with feat on partitions), so aggregation results come out
    of PSUM already in the layout the next layer consumes.
  - Layer 1 does NO gather at all: the host pre-permutes x[src]*dis[src]
    into edge-slot tiles (xp input). Each 128-slot tile is consumed directly
    as matmul weights (lhsT): psum[feat, dst] += Xp_tile^T-contract @ S_tile,
    where S is the host-built one-hot slot->dst-in-block matrix (fp8,
    streamed from DRAM each layer). The self-loop term is one more matmul
    into the same PSUM: lhsT=W, rhs=x^T*dis (xds input).
  - Layers 2/3: per layer, the (activations @ W)*dis table [50176, 128] bf16
    is replicated via AllGather, then per-edge rows are fetched with
    gpsimd.dma_gather. Gathers are spread round-robin over 4 SWDGE queues —
    descriptor generation for queues 1-3 runs asynchronously on other Q7
    contexts, ~4x the single-queue rate. Table is addressed as two 25088-row
    halves (int16 gather indices). Gathered tiles feed the same S-matmul
    accumulation (gathered tile as lhsT, fp8 S as rhs).
  - Epilogue per block: zd = psum * dis[dst] (DVE), hT = relu(zd) (ACT
    engine), hdd = hT * dis (DVE, self rhs for the next layer), stage-A
    for the next table fused here (matmul, *dis, PE-transpose, copy, DMA).
  - mu and logstd heads share one propagation via [Wmu|Wls] concat; the
    output is written feature-major and transposed on host.
  - All index preprocessing host-side; bf16 storage/matmul, fp32 PSUM.
"""

import numpy as np
import ml_dtypes

import concourse.mybir as mybir
import concourse.tile as tile
from concourse import bacc
from concourse import library_config
from concourse.bass_utils import run_bass_kernel_spmd

P = 128
NCORE = 8
N = 50000
NOWN = N // NCORE            # 6250 nodes per core
NB = (NOWN + P - 1) // P     # 49 blocks
NPAD = NB * P                # 6272
VROWS = NCORE * NPAD         # 50176 table rows
VHALF = VROWS // 2           # 25088 (< 2^15, int16-addressable)
KH = 10                      # edge tiles per block per table half
KT = 2 * KH                  # 20 tiles per block total
# Bulk gathers need single_packet=False: with the default True, every
# descriptor must fit one packet per SDMA engine (<=64), and >1024 indices
# hard-crashes the device (NRT_EXEC_UNIT_UNRECOVERABLE).
# GB=1 (one gather per block-half) so real edges form a prefix of each idx
# panel: trailing -1 indices + a runtime count register make the Q7 skip all
# ghost-pad descriptors (~20% of rows).
HB = 25                      # blocks in the first AllGather half
HROWS = HB * P               # 3200 rows
GB = 1
NG = (NB + GB - 1) // GB     # 49 gather groups
GSLOT = GB * KH * P          # idx slots per gather (1280)
NQ = 4                       # SWDGE queues: desc-gen contexts run in parallel

_bf = mybir.dt.bfloat16
_f32 = mybir.dt.float32
_i16 = mybir.dt.int16
_fp8 = mybir.dt.float8e4
_i32 = mybir.dt.int32
_bf_np = ml_dtypes.bfloat16
_fp8_np = ml_dtypes.float8_e4m3

TRACE = False        # set by test harness for profiling runs
TRACE_DIR = None

_cache = {}


def _build_program(use_bias: bool):
    nc = bacc.Bacc("TRN2", num_devices=NCORE, debug=False, num_swdge_queues=NQ)

    Wc = nc.dram_tensor("Wc", [P, 3 * P], _bf, kind="ExternalInput")
    disr = nc.dram_tensor("disr", [P, NPAD], _f32, kind="ExternalInput")
    xds = nc.dram_tensor("xds", [P, NPAD], _bf, kind="ExternalInput")
    ident = nc.dram_tensor("ident", [P, P], _bf, kind="ExternalInput")
    xp = nc.dram_tensor("xp", [P, NB * KT * P], _bf, kind="ExternalInput")
    sS = nc.dram_tensor("sS", [NB * P, KT * P], _fp8, kind="ExternalInput")
    # wrapped int16 gather indices, one [P, GSLOT//16] panel per group per half
    idxAB = nc.dram_tensor("idxAB", [P, 2 * NG * (GSLOT // 16)], _i16,
                           kind="ExternalInput")
    ncnt = nc.dram_tensor("ncnt", [1, 2 * NG], _i32, kind="ExternalInput")
    if use_bias:
        brep = nc.dram_tensor("brep", [P, 3], _f32, kind="ExternalInput")
    outf = nc.dram_tensor("outf", [P, NPAD], _f32, kind="ExternalOutput")
    shard = nc.dram_tensor("shard", [NPAD, P], _bf)
    tableA = nc.dram_tensor("tableA", [NCORE * HROWS, P], _bf,
                            addr_space="Shared")
    tableB = nc.dram_tensor("tableB", [NCORE * (NPAD - HROWS), P], _bf,
                            addr_space="Shared")
    tableA2 = nc.dram_tensor("tableA2", [NCORE * HROWS, P], _bf,
                             addr_space="Shared")
    tableB2 = nc.dram_tensor("tableB2", [NCORE * (NPAD - HROWS), P], _bf,
                             addr_space="Shared")
    wrmin = nc.dram_tensor("wrmin", [1, 16], _bf)
    wrmout = nc.dram_tensor("wrmout", [NCORE, 16], _bf, addr_space="Shared")

    with tile.TileContext(nc) as tc:
        with tc.tile_pool(name="meta", bufs=1) as meta, \
             tc.tile_pool(name="sb", bufs=4) as sb, \
             tc.tile_pool(name="xpp", bufs=4) as xpp, \
             tc.tile_pool(name="mg", bufs=8) as mg, \
             tc.tile_pool(name="ps", bufs=2, space="PSUM") as ps:
            nc.gpsimd.load_library(library_config.mlp)
            # tiny dummy AllGather: warms up the collective rings so the
            # first real AllGather doesn't pay ~85us of setup
            nc.gpsimd.collective_compute(
                "AllGather", mybir.AluOpType.bypass,
                replica_groups=[list(range(NCORE))],
                ins=[wrmin[:]], outs=[wrmout[:]])
            Wc_s = meta.tile([P, 3 * P], _bf)
            nc.sync.dma_start(Wc_s[:], Wc[:])
            ident_s = meta.tile([P, P], _bf)
            nc.sync.dma_start(ident_s[:], ident[:])
            idx_s = meta.tile([P, 2 * NG * (GSLOT // 16)], _i16)
            nc.sync.dma_start(idx_s[:], idxAB[:])
            cnt_s = meta.tile([1, 2 * NG], _i32)
            nc.sync.dma_start(cnt_s[:], ncnt[:])
            disr_s = meta.tile([P, NPAD], _f32)
            nc.sync.dma_start(disr_s[:], disr[:])
            xds_s = meta.tile([P, NPAD], _bf)
            nc.sync.dma_start(xds_s[:], xds[:])
            if use_bias:
                br_s = meta.tile([P, 3], _f32)
                nc.sync.dma_start(br_s[:], brep[:])

            hT_s = meta.tile([P, NPAD], _bf)    # activations, [feat, node]
            hdd_s = meta.tile([P, NPAD], _bf)   # activations * dis^2

            def epilogue(l, b, pb):
                """pb: PSUM [feat, dst-node] pre-norm aggregate for block b."""
                bs = slice(b * P, (b + 1) * P)
                if l < 2:
                    zd = sb.tile([P, P], _f32, tag="zd")
                    nc.vector.tensor_tensor(
                        out=zd[:], in0=pb[:], in1=disr_s[:, bs],
                        op=mybir.AluOpType.mult)
                    if use_bias:
                        nc.vector.tensor_scalar_add(
                            zd[:], zd[:], br_s[:, l:l + 1])
                    nc.scalar.activation(
                        hT_s[:, bs], zd[:],
                        mybir.ActivationFunctionType.Relu)
                    nc.vector.tensor_tensor(
                        out=hdd_s[:, bs], in0=hT_s[:, bs],
                        in1=disr_s[:, bs], op=mybir.AluOpType.mult)
                    # stage A: next layer's table rows for this block
                    py = ps.tile([P, P], _f32, tag="py")
                    nc.tensor.matmul(py[:], lhsT=Wc_s[:, (l + 1) * P:(l + 2) * P],
                                     rhs=hT_s[:, bs], start=True, stop=True)
                    yh = sb.tile([P, P], _bf, tag="yh")
                    nc.vector.tensor_tensor(
                        out=yh[:], in0=py[:], in1=disr_s[:, bs],
                        op=mybir.AluOpType.mult)
                    pt = ps.tile([P, P], _bf, tag="pt")
                    nc.tensor.transpose(pt[:], yh[:], ident_s[:])
                    sh = sb.tile([P, P], _bf, tag="sh")
                    nc.scalar.copy(sh[:], pt[:])
                    nc.sync.dma_start(shard[bs, :], sh[:])
                else:
                    of = sb.tile([P, P], _f32, tag="of")
                    nc.vector.tensor_tensor(
                        out=of[:], in0=pb[:], in1=disr_s[:, bs],
                        op=mybir.AluOpType.mult)
                    if use_bias:
                        nc.vector.tensor_scalar_add(
                            of[:], of[:], br_s[:, 2:3])
                    nc.sync.dma_start(outf[:, bs], of[:])

            def all_gather_a(dst):
                nc.gpsimd.collective_compute(
                    "AllGather", mybir.AluOpType.bypass,
                    replica_groups=[list(range(NCORE))],
                    ins=[shard[0:HROWS, :]], outs=[dst[:]])

            def all_gather_b(dst):
                nc.gpsimd.collective_compute(
                    "AllGather", mybir.AluOpType.bypass,
                    replica_groups=[list(range(NCORE))],
                    ins=[shard[HROWS:NPAD, :]], outs=[dst[:]])

            # ---- layer 1: no gather; host-permuted Xp tiles as weights.
            # Aggregate raw x*dis rows (incl. self via identity matmul) in
            # input space, then apply W1 once per block: the aggregation is
            # linear, so agg(xW) = agg(x) @ W.
            for b in range(NB):
                S = sb.tile([P, KT * P], _fp8, tag="S")
                nc.sync.dma_start(S[:], sS[b * P:(b + 1) * P, :])
                Xp = xpp.tile([P, KT, P], _bf, tag="Xp")
                nc.sync.dma_start(
                    Xp[:], xp[:, b * KT * P:(b + 1) * KT * P]
                    .rearrange("p (t f) -> p t f", t=KT))
                px = ps.tile([P, P], _f32, tag="px")
                for t in range(KT):
                    nc.tensor.matmul(px[:], lhsT=Xp[:, t, :],
                                     rhs=S[:, t * P:(t + 1) * P],
                                     start=(t == 0), stop=False)
                nc.tensor.matmul(px[:], lhsT=ident_s[:],
                                 rhs=xds_s[:, b * P:(b + 1) * P],
                                 start=False, stop=True)
                ax = sb.tile([P, P], _bf, tag="ax")
                nc.scalar.copy(ax[:], px[:])
                pb = ps.tile([P, P], _f32, tag="pb")
                nc.tensor.matmul(pb[:], lhsT=Wc_s[:, 0:P], rhs=ax[:],
                                 start=True, stop=True)
                epilogue(0, b, pb)
                if b == HB - 1:
                    all_gather_a(tableA)
                elif b == NB - 1:
                    all_gather_b(tableB)


            # ---- layers 2/3: gather-based aggregation ----
            rc_cm = nc.gpsimd.register("gcnt")
            rc = rc_cm.__enter__()
            for l in range(1, 3):
                for g in range(NG):
                    b = g
                    MA = mg.tile([P, KH, P], _bf, tag="MA")
                    MB = mg.tile([P, KH, P], _bf, tag="MB")
                    if l == 1 and g < 8:
                        # first touch of each pool slot: clear stale SBUF so
                        # count-skipped tail slots hold 0, not garbage (0*Inf
                        # would poison the S-masked matmul)
                        nc.vector.memset(MA[:], 0)
                        nc.vector.memset(MB[:], 0)
                    cA = (2 * g) * (GSLOT // 16)
                    cB = (2 * g + 1) * (GSLOT // 16)
                    nc.gpsimd.reg_load(rc, cnt_s[0:1, 2 * g:2 * g + 1])
                    nc.gpsimd.dma_gather(
                        MA[:], (tableA if l == 1 else tableA2)[:, :],
                        idx_s[:, cA:cA + GSLOT // 16], GSLOT, rc, P,
                        single_packet=False, queue_num=(2 * g) % NQ)
                    nc.gpsimd.reg_load(rc, cnt_s[0:1, 2 * g + 1:2 * g + 2])
                    nc.gpsimd.dma_gather(
                        MB[:], (tableB if l == 1 else tableB2)[:, :],
                        idx_s[:, cB:cB + GSLOT // 16], GSLOT, rc, P,
                        single_packet=False, queue_num=(2 * g + 1) % NQ)
                    S = sb.tile([P, KT * P], _fp8, tag="S")
                    nc.sync.dma_start(S[:], sS[b * P:(b + 1) * P, :])
                    pb = ps.tile([P, P], _f32, tag="pb")
                    for t in range(KT):
                        Msrc = MA if t < KH else MB
                        nc.tensor.matmul(pb[:], lhsT=Msrc[:, t % KH, :],
                                         rhs=S[:, t * P:(t + 1) * P],
                                         start=(t == 0), stop=False)
                    nc.tensor.matmul(pb[:], lhsT=Wc_s[:, l * P:(l + 1) * P],
                                     rhs=hdd_s[:, b * P:(b + 1) * P],
                                     start=False, stop=True)
                    epilogue(l, b, pb)
                    if l == 1 and b == HB - 1:
                        all_gather_a(tableA2)
                    elif l == 1 and b == NB - 1:
                        all_gather_b(tableB2)
    nc.compile()
    return nc


def _wrap_idx(idx_flat):
    """dma_gather wrapped layout: slot j at [j%16, j//16], replicated over the
    8 groups of 16 partitions."""
    w = idx_flat.reshape(-1, 16).T          # [16, slots//16]
    return np.tile(w, (8, 1)).astype(np.int16)


def _preprocess(x, edge_index, W1, b1, W2, b2, Wmu, bmu, Wls, bls):
    src_g = np.asarray(edge_index[0]).astype(np.int64)
    dst_g = np.asarray(edge_index[1]).astype(np.int64)
    x = np.asarray(x, dtype=np.float32)

    deg = (np.bincount(dst_g, minlength=N) + 1).astype(np.float32)
    dis = (1.0 / np.sqrt(deg)).astype(np.float32)

    src_core = src_g // NOWN
    rloc = src_g - src_core * NOWN          # row within owning core
    dst_core = dst_g // NOWN

    Wmh = np.concatenate([np.asarray(Wmu), np.asarray(Wls)], axis=1)
    Wc_np = np.concatenate(
        [np.asarray(W1), np.asarray(W2), Wmh], axis=1).astype(_bf_np)
    bmh = np.concatenate([np.asarray(bmu), np.asarray(bls)])
    ball = np.stack([np.asarray(b1), np.asarray(b2), bmh], axis=1)
    use_bias = bool(np.any(ball != 0.0))
    brep_np = ball.astype(np.float32)       # [P, 3]

    ident_np = np.eye(P, dtype=np.float32).astype(_bf_np)
    xsrc = x[src_g] * dis[src_g][:, None]   # per-edge premultiplied source

    in_maps = []
    for c in range(NCORE):
        sel = dst_core == c
        dl = dst_g[sel] - c * NOWN
        rl = rloc[sel]
        sc = src_core[sel]
        half = (rl >= HROWS).astype(np.int64)
        trh = np.where(half == 0, sc * HROWS + rl,
                       sc * (NPAD - HROWS) + (rl - HROWS))
        blocks = dl >> 7
        loc = (dl & 127).astype(np.int64)

        # order by (block, half), then pack each (block, half) bucket into its
        # fixed KH*P slot range
        keys = blocks * 2 + half
        order = np.argsort(keys, kind="stable")
        ksort = keys[order]
        counts = np.bincount(ksort, minlength=2 * NB)
        assert counts.max() <= KH * P, f"block-half overflow: {counts.max()}"
        starts = np.zeros(2 * NB, np.int64)
        starts[1:] = np.cumsum(counts)[:-1]
        pos = np.arange(len(ksort)) - starts[ksort]

        kb = ksort >> 1
        kh = ksort & 1

        # gather idx panels: real edges form a prefix (GB=1), trailing
        # ghosts are -1 and skipped via the runtime count register
        idx_flat = np.full((2 * NG, GSLOT), -1, np.int64)
        idx_flat[2 * kb + kh, pos] = trh[order]
        assert counts.min() >= 1, "empty block-half"
        ncnt_np = counts.astype(np.int32)[None, :]
        idx_panels = np.concatenate(
            [_wrap_idx(idx_flat[i]) for i in range(2 * NG)], axis=1)

        # host-built one-hot S: per block b, S[b*P + slot_part,
        # tile*P + dst_in_block] = 1
        tile_in_b = kh * KH + (pos >> 7)
        prt = pos & 127
        S_np = np.zeros((NB * P, KT * P), np.float32)
        S_np[kb * P + prt, tile_in_b * P + loc[order]] = 1.0

        # host-permuted layer-1 messages: xp[slot_part,
        # (block*KT + tile)*P : +P] = x[src]*dis[src]
        xp3 = np.zeros((P, NB * KT, P), np.float32)
        xp3[prt, kb * KT + tile_in_b, :] = xsrc[sel][order]

        dpad = np.zeros((NPAD,), np.float32)
        dpad[:NOWN] = dis[c * NOWN:(c + 1) * NOWN]
        disr_np = np.tile(dpad[None, :], (P, 1)).astype(np.float32)
        xds_np = (x[c * NOWN:(c + 1) * NOWN]
                  * dpad[:NOWN, None]).T
        xds_full = np.zeros((P, NPAD), np.float32)
        xds_full[:, :NOWN] = xds_np

        im = dict(
            Wc=Wc_np,
            disr=disr_np,
            xds=xds_full.astype(_bf_np),
            ident=ident_np,
            xp=xp3.reshape(P, NB * KT * P).astype(_bf_np),
            sS=S_np.astype(_fp8_np),
            idxAB=idx_panels,
            ncnt=ncnt_np,
        )
        if use_bias:
            im["brep"] = brep_np
        in_maps.append(im)
    return in_maps, use_bias


def kernel(x, edge_index, W1, b1, W2, b2, Wmu, bmu, Wls, bls):
    in_maps, use_bias = _preprocess(
        x, edge_index, W1, b1, W2, b2, Wmu, bmu, Wls, bls)
    if use_bias not in _cache:
        _cache[use_bias] = _build_program(use_bias)
    nc = _cache[use_bias]
    kwargs = {}
    if TRACE:
        kwargs = dict(trace=True, tmpdir=TRACE_DIR)
    res = run_bass_kernel_spmd(nc, in_maps, list(range(NCORE)), **kwargs)
    if TRACE:
        globals()["LAST_RESULT"] = res
    out = np.concatenate(
        [res.results[c]["outf"][:, :NOWN].T for c in range(NCORE)], axis=0)
    mu = np.ascontiguousarray(out[:, :64], dtype=np.float32)
    logstd = np.ascontiguousarray(out[:, 64:], dtype=np.float32)
    return (mu, logstd)


# revision 15
# speedup vs baseline: 1.1221x; 1.1221x over previous
"""GCN encoder (2x GCNConv+ReLU, then fused mu/logstd heads) on 8 Trainium2
NeuronCores, Bass/Tile SPMD.

Strategy (node-parallel, feature-major pipeline):
  - Nodes sharded by range: core c owns rows [c*6250, (c+1)*6250), padded to
    6272 = 49 blocks of 128. All on-chip activations are FEATURE-major
    ([feat, node] with feat on partitions), so aggregation results come out
    of PSUM already in the layout the next layer consumes.
  - Layer 1 does NO gather at all: the host pre-permutes x[src]*dis[src]
    into edge-slot tiles (xp input). Each 128-slot tile is consumed directly
    as matmul weights (lhsT): psum[feat, dst] += Xp_tile^T-contract @ S_tile,
    where S is the host-built one-hot slot->dst-in-block matrix (fp8,
    streamed from DRAM each layer). The self-loop term is one more matmul
    into the same PSUM: lhsT=W, rhs=x^T*dis (xds input).
  - Layers 2/3: per layer, the (activations @ W)*dis table [50176, 128] bf16
    is replicated via AllGather, then per-edge rows are fetched with
    gpsimd.dma_gather. Gathers are spread round-robin over 4 SWDGE queues —
    descriptor generation for queues 1-3 runs asynchronously on other Q7
    contexts, ~4x the single-queue rate. Table is addressed as two 25088-row
    halves (int16 gather indices). Gathered tiles feed the same S-matmul
    accumulation (gathered tile as lhsT, fp8 S as rhs).
  - Epilogue per block: zd = psum * dis[dst] (DVE), hT = relu(zd) (ACT
    engine), hdd = hT * dis (DVE, self rhs for the next layer), stage-A
    for the next table fused here (matmul, *dis, PE-transpose, copy, DMA).
  - mu and logstd heads share one propagation via [Wmu|Wls] concat; the
    output is written feature-major and transposed on host.
  - All index preprocessing host-side; bf16 storage/matmul, fp32 PSUM.
"""

import numpy as np
import ml_dtypes

import concourse.mybir as mybir
import concourse.tile as tile
from concourse import bacc
from concourse import library_config
from concourse.bass_utils import run_bass_kernel_spmd

P = 128
NCORE = 8
N = 50000
NOWN = N // NCORE            # 6250 nodes per core
NB = (NOWN + P - 1) // P     # 49 blocks
NPAD = NB * P                # 6272
VROWS = NCORE * NPAD         # 50176 table rows
VHALF = VROWS // 2           # 25088 (< 2^15, int16-addressable)
KH = 10                      # edge tiles per block per table half
KT = 2 * KH                  # 20 tiles per block total
# Bulk gathers need single_packet=False: with the default True, every
# descriptor must fit one packet per SDMA engine (<=64), and >1024 indices
# hard-crashes the device (NRT_EXEC_UNIT_UNRECOVERABLE).
# GB=1 (one gather per block-half) so real edges form a prefix of each idx
# panel: trailing -1 indices + a runtime count register make the Q7 skip all
# ghost-pad descriptors (~20% of rows).
HB = 25                      # blocks in the first AllGather half
HROWS = HB * P               # 3200 rows
GB = 1
NG = (NB + GB - 1) // GB     # 49 gather groups
GSLOT = GB * KH * P          # idx slots per gather (1280)
NQ = 4                       # SWDGE queues: desc-gen contexts run in parallel

_bf = mybir.dt.bfloat16
_f32 = mybir.dt.float32
_i16 = mybir.dt.int16
_fp8 = mybir.dt.float8e4
_i32 = mybir.dt.int32
_bf_np = ml_dtypes.bfloat16
_fp8_np = ml_dtypes.float8_e4m3

TRACE = False        # set by test harness for profiling runs
TRACE_DIR = None

_cache = {}


def _build_program(use_bias: bool):
    nc = bacc.Bacc("TRN2", num_devices=NCORE, debug=False, num_swdge_queues=NQ)

    Wc = nc.dram_tensor("Wc", [P, 3 * P], _bf, kind="ExternalInput")
    disr = nc.dram_tensor("disr", [P, NPAD], _f32, kind="ExternalInput")
    xds = nc.dram_tensor("xds", [P, NPAD], _bf, kind="ExternalInput")
    ident = nc.dram_tensor("ident", [P, P], _bf, kind="ExternalInput")
    xp = nc.dram_tensor("xp", [P, NB * KT * P], _bf, kind="ExternalInput")
    sS = nc.dram_tensor("sS", [NB * P, KT * P], _fp8, kind="ExternalInput")
    # wrapped int16 gather indices, one [P, GSLOT//16] panel per group per half
    idxAB = nc.dram_tensor("idxAB", [P, 2 * NG * (GSLOT // 16)], _i16,
                           kind="ExternalInput")
    ncnt = nc.dram_tensor("ncnt", [1, 2 * NG], _i32, kind="ExternalInput")
    if use_bias:
        brep = nc.dram_tensor("brep", [P, 3], _f32, kind="ExternalInput")
    outf = nc.dram_tensor("outf", [P, NPAD], _f32, kind="ExternalOutput")
    shard = nc.dram_tensor("shard", [NPAD, P], _bf)
    table = nc.dram_tensor("table", [VROWS, P], _bf, addr_space="Shared")
    wrmin = nc.dram_tensor("wrmin", [1, 16], _bf)
    wrmout = nc.dram_tensor("wrmout", [NCORE, 16], _bf, addr_space="Shared")

    with tile.TileContext(nc) as tc:
        with tc.tile_pool(name="meta", bufs=1) as meta, \
             tc.tile_pool(name="sb", bufs=4) as sb, \
             tc.tile_pool(name="xpp", bufs=4) as xpp, \
             tc.tile_pool(name="mg", bufs=8) as mg, \
             tc.tile_pool(name="ps", bufs=2, space="PSUM") as ps:
            nc.gpsimd.load_library(library_config.mlp)
            # tiny dummy AllGather: warms up the collective rings so the
            # first real AllGather doesn't pay ~85us of setup
            nc.gpsimd.collective_compute(
                "AllGather", mybir.AluOpType.bypass,
                replica_groups=[list(range(NCORE))],
                ins=[wrmin[:]], outs=[wrmout[:]])
            Wc_s = meta.tile([P, 3 * P], _bf)
            nc.sync.dma_start(Wc_s[:], Wc[:])
            ident_s = meta.tile([P, P], _bf)
            nc.sync.dma_start(ident_s[:], ident[:])
            idx_s = meta.tile([P, 2 * NG * (GSLOT // 16)], _i16)
            nc.sync.dma_start(idx_s[:], idxAB[:])
            cnt_s = meta.tile([1, 2 * NG], _i32)
            nc.sync.dma_start(cnt_s[:], ncnt[:])
            disr_s = meta.tile([P, NPAD], _f32)
            nc.sync.dma_start(disr_s[:], disr[:])
            xds_s = meta.tile([P, NPAD], _bf)
            nc.sync.dma_start(xds_s[:], xds[:])
            if use_bias:
                br_s = meta.tile([P, 3], _f32)
                nc.sync.dma_start(br_s[:], brep[:])

            hT_s = meta.tile([P, NPAD], _bf)    # activations, [feat, node]
            hdd_s = meta.tile([P, NPAD], _bf)   # activations * dis^2

            def epilogue(l, b, pb):
                """pb: PSUM [feat, dst-node] pre-norm aggregate for block b."""
                bs = slice(b * P, (b + 1) * P)
                if l < 2:
                    zd = sb.tile([P, P], _f32, tag="zd")
                    nc.vector.tensor_tensor(
                        out=zd[:], in0=pb[:], in1=disr_s[:, bs],
                        op=mybir.AluOpType.mult)
                    if use_bias:
                        nc.vector.tensor_scalar_add(
                            zd[:], zd[:], br_s[:, l:l + 1])
                    nc.scalar.activation(
                        hT_s[:, bs], zd[:],
                        mybir.ActivationFunctionType.Relu)
                    nc.vector.tensor_tensor(
                        out=hdd_s[:, bs], in0=hT_s[:, bs],
                        in1=disr_s[:, bs], op=mybir.AluOpType.mult)
                    # stage A: next layer's table rows for this block
                    py = ps.tile([P, P], _f32, tag="py")
                    nc.tensor.matmul(py[:], lhsT=Wc_s[:, (l + 1) * P:(l + 2) * P],
                                     rhs=hT_s[:, bs], start=True, stop=True)
                    yh = sb.tile([P, P], _bf, tag="yh")
                    nc.vector.tensor_tensor(
                        out=yh[:], in0=py[:], in1=disr_s[:, bs],
                        op=mybir.AluOpType.mult)
                    pt = ps.tile([P, P], _bf, tag="pt")
                    nc.tensor.transpose(pt[:], yh[:], ident_s[:])
                    sh = sb.tile([P, P], _bf, tag="sh")
                    nc.scalar.copy(sh[:], pt[:])
                    nc.sync.dma_start(shard[bs, :], sh[:])
                else:
                    of = sb.tile([P, P], _f32, tag="of")
                    nc.vector.tensor_tensor(
                        out=of[:], in0=pb[:], in1=disr_s[:, bs],
                        op=mybir.AluOpType.mult)
                    if use_bias:
                        nc.vector.tensor_scalar_add(
                            of[:], of[:], br_s[:, 2:3])
                    nc.sync.dma_start(outf[:, bs], of[:])

            def all_gather():
                nc.gpsimd.collective_compute(
                    "AllGather", mybir.AluOpType.bypass,
                    replica_groups=[list(range(NCORE))],
                    ins=[shard[:]], outs=[table[:]])

            # ---- layer 1: no gather; host-permuted Xp tiles as weights.
            # Aggregate raw x*dis rows (incl. self via identity matmul) in
            # input space, then apply W1 once per block: the aggregation is
            # linear, so agg(xW) = agg(x) @ W.
            for b in range(NB):
                S = sb.tile([P, KT * P], _fp8, tag="S")
                nc.sync.dma_start(S[:], sS[b * P:(b + 1) * P, :])
                Xp = xpp.tile([P, KT, P], _bf, tag="Xp")
                nc.sync.dma_start(
                    Xp[:], xp[:, b * KT * P:(b + 1) * KT * P]
                    .rearrange("p (t f) -> p t f", t=KT))
                px = ps.tile([P, P], _f32, tag="px")
                for t in range(KT):
                    nc.tensor.matmul(px[:], lhsT=Xp[:, t, :],
                                     rhs=S[:, t * P:(t + 1) * P],
                                     start=(t == 0), stop=False)
                nc.tensor.matmul(px[:], lhsT=ident_s[:],
                                 rhs=xds_s[:, b * P:(b + 1) * P],
                                 start=False, stop=True)
                ax = sb.tile([P, P], _bf, tag="ax")
                nc.scalar.copy(ax[:], px[:])
                pb = ps.tile([P, P], _f32, tag="pb")
                nc.tensor.matmul(pb[:], lhsT=Wc_s[:, 0:P], rhs=ax[:],
                                 start=True, stop=True)
                epilogue(0, b, pb)

            all_gather()


            # ---- layers 2/3: gather-based aggregation ----
            rc_cm = nc.gpsimd.register("gcnt")
            rc = rc_cm.__enter__()
            for l in range(1, 3):
                for g in range(NG):
                    b = g
                    MA = mg.tile([P, KH, P], _bf, tag="MA")
                    MB = mg.tile([P, KH, P], _bf, tag="MB")
                    if l == 1 and g < 8:
                        # first touch of each pool slot: clear stale SBUF so
                        # count-skipped tail slots hold 0, not garbage (0*Inf
                        # would poison the S-masked matmul)
                        nc.vector.memset(MA[:], 0)
                        nc.vector.memset(MB[:], 0)
                    cA = (2 * g) * (GSLOT // 16)
                    cB = (2 * g + 1) * (GSLOT // 16)
                    nc.gpsimd.reg_load(rc, cnt_s[0:1, 2 * g:2 * g + 1])
                    nc.gpsimd.dma_gather(
                        MA[:], table[0:VHALF, :],
                        idx_s[:, cA:cA + GSLOT // 16], GSLOT, rc, P,
                        single_packet=False, queue_num=(2 * g) % NQ)
                    nc.gpsimd.reg_load(rc, cnt_s[0:1, 2 * g + 1:2 * g + 2])
                    nc.gpsimd.dma_gather(
                        MB[:], table[VHALF:VROWS, :],
                        idx_s[:, cB:cB + GSLOT // 16], GSLOT, rc, P,
                        single_packet=False, queue_num=(2 * g + 1) % NQ)
                    S = sb.tile([P, KT * P], _fp8, tag="S")
                    nc.sync.dma_start(S[:], sS[b * P:(b + 1) * P, :])
                    pb = ps.tile([P, P], _f32, tag="pb")
                    for t in range(KT):
                        Msrc = MA if t < KH else MB
                        nc.tensor.matmul(pb[:], lhsT=Msrc[:, t % KH, :],
                                         rhs=S[:, t * P:(t + 1) * P],
                                         start=(t == 0), stop=False)
                    nc.tensor.matmul(pb[:], lhsT=Wc_s[:, l * P:(l + 1) * P],
                                     rhs=hdd_s[:, b * P:(b + 1) * P],
                                     start=False, stop=True)
                    epilogue(l, b, pb)
                if l == 1:
                    all_gather()
    nc.compile()
    return nc


def _wrap_idx(idx_flat):
    """dma_gather wrapped layout: slot j at [j%16, j//16], replicated over the
    8 groups of 16 partitions."""
    w = idx_flat.reshape(-1, 16).T          # [16, slots//16]
    return np.tile(w, (8, 1)).astype(np.int16)


def _preprocess(x, edge_index, W1, b1, W2, b2, Wmu, bmu, Wls, bls):
    src_g = np.asarray(edge_index[0]).astype(np.int64)
    dst_g = np.asarray(edge_index[1]).astype(np.int64)
    x = np.asarray(x, dtype=np.float32)

    deg = (np.bincount(dst_g, minlength=N) + 1).astype(np.float32)
    dis = (1.0 / np.sqrt(deg)).astype(np.float32)

    src_core = src_g // NOWN
    tabrow = (src_core * NPAD + (src_g - src_core * NOWN)).astype(np.int64)
    dst_core = dst_g // NOWN

    Wmh = np.concatenate([np.asarray(Wmu), np.asarray(Wls)], axis=1)
    Wc_np = np.concatenate(
        [np.asarray(W1), np.asarray(W2), Wmh], axis=1).astype(_bf_np)
    bmh = np.concatenate([np.asarray(bmu), np.asarray(bls)])
    ball = np.stack([np.asarray(b1), np.asarray(b2), bmh], axis=1)
    use_bias = bool(np.any(ball != 0.0))
    brep_np = ball.astype(np.float32)       # [P, 3]

    ident_np = np.eye(P, dtype=np.float32).astype(_bf_np)
    xsrc = x[src_g] * dis[src_g][:, None]   # per-edge premultiplied source

    in_maps = []
    for c in range(NCORE):
        sel = dst_core == c
        dl = dst_g[sel] - c * NOWN
        tr = tabrow[sel]
        half = (tr >= VHALF).astype(np.int64)
        trh = tr - half * VHALF            # row within half, < 25088
        blocks = dl >> 7
        loc = (dl & 127).astype(np.int64)

        # order by (block, half), then pack each (block, half) bucket into its
        # fixed KH*P slot range
        keys = blocks * 2 + half
        order = np.argsort(keys, kind="stable")
        ksort = keys[order]
        counts = np.bincount(ksort, minlength=2 * NB)
        assert counts.max() <= KH * P, f"block-half overflow: {counts.max()}"
        starts = np.zeros(2 * NB, np.int64)
        starts[1:] = np.cumsum(counts)[:-1]
        pos = np.arange(len(ksort)) - starts[ksort]

        kb = ksort >> 1
        kh = ksort & 1

        # gather idx panels: real edges form a prefix (GB=1), trailing
        # ghosts are -1 and skipped via the runtime count register
        idx_flat = np.full((2 * NG, GSLOT), -1, np.int64)
        idx_flat[2 * kb + kh, pos] = trh[order]
        assert counts.min() >= 1, "empty block-half"
        ncnt_np = counts.astype(np.int32)[None, :]
        idx_panels = np.concatenate(
            [_wrap_idx(idx_flat[i]) for i in range(2 * NG)], axis=1)

        # host-built one-hot S: per block b, S[b*P + slot_part,
        # tile*P + dst_in_block] = 1
        tile_in_b = kh * KH + (pos >> 7)
        prt = pos & 127
        S_np = np.zeros((NB * P, KT * P), np.float32)
        S_np[kb * P + prt, tile_in_b * P + loc[order]] = 1.0

        # host-permuted layer-1 messages: xp[slot_part,
        # (block*KT + tile)*P : +P] = x[src]*dis[src]
        xp3 = np.zeros((P, NB * KT, P), np.float32)
        xp3[prt, kb * KT + tile_in_b, :] = xsrc[sel][order]

        dpad = np.zeros((NPAD,), np.float32)
        dpad[:NOWN] = dis[c * NOWN:(c + 1) * NOWN]
        disr_np = np.tile(dpad[None, :], (P, 1)).astype(np.float32)
        xds_np = (x[c * NOWN:(c + 1) * NOWN]
                  * dpad[:NOWN, None]).T
        xds_full = np.zeros((P, NPAD), np.float32)
        xds_full[:, :NOWN] = xds_np

        im = dict(
            Wc=Wc_np,
            disr=disr_np,
            xds=xds_full.astype(_bf_np),
            ident=ident_np,
            xp=xp3.reshape(P, NB * KT * P).astype(_bf_np),
            sS=S_np.astype(_fp8_np),
            idxAB=idx_panels,
            ncnt=ncnt_np,
        )
        if use_bias:
            im["brep"] = brep_np
        in_maps.append(im)
    return in_maps, use_bias


def kernel(x, edge_index, W1, b1, W2, b2, Wmu, bmu, Wls, bls):
    in_maps, use_bias = _preprocess(
        x, edge_index, W1, b1, W2, b2, Wmu, bmu, Wls, bls)
    if use_bias not in _cache:
        _cache[use_bias] = _build_program(use_bias)
    nc = _cache[use_bias]
    kwargs = {}
    if TRACE:
        kwargs = dict(trace=True, tmpdir=TRACE_DIR)
    res = run_bass_kernel_spmd(nc, in_maps, list(range(NCORE)), **kwargs)
    if TRACE:
        globals()["LAST_RESULT"] = res
    out = np.concatenate(
        [res.results[c]["outf"][:, :NOWN].T for c in range(NCORE)], axis=0)
    mu = np.ascontiguousarray(out[:, :64], dtype=np.float32)
    logstd = np.ascontiguousarray(out[:, 64:], dtype=np.float32)
    return (mu, logstd)


# revision 16
# speedup vs baseline: 1.1637x; 1.0370x over previous
"""GCN encoder (2x GCNConv+ReLU, then fused mu/logstd heads) on 8 Trainium2
NeuronCores, Bass/Tile SPMD.

Strategy (node-parallel, feature-major pipeline):
  - Nodes sharded by range: core c owns rows [c*6250, (c+1)*6250), padded to
    6272 = 49 blocks of 128. All on-chip activations are FEATURE-major
    ([feat, node] with feat on partitions), so aggregation results come out
    of PSUM already in the layout the next layer consumes.
  - Layer 1 does NO gather at all: the host pre-permutes x[src]*dis[src]
    into edge-slot tiles (xp input). Each 128-slot tile is consumed directly
    as matmul weights (lhsT): psum[feat, dst] += Xp_tile^T-contract @ S_tile,
    where S is the host-built one-hot slot->dst-in-block matrix (fp8,
    streamed from DRAM each layer). The self-loop term is one more matmul
    into the same PSUM: lhsT=W, rhs=x^T*dis (xds input).
  - Layers 2/3: per layer, the (activations @ W)*dis table [50176, 128] bf16
    is replicated via AllGather, then per-edge rows are fetched with
    gpsimd.dma_gather. Gathers are spread round-robin over 4 SWDGE queues —
    descriptor generation for queues 1-3 runs asynchronously on other Q7
    contexts, ~4x the single-queue rate. Table is addressed as two 25088-row
    halves (int16 gather indices). Gathered tiles feed the same S-matmul
    accumulation (gathered tile as lhsT, fp8 S as rhs).
  - Epilogue per block: zd = psum * dis[dst] (DVE), hT = relu(zd) (ACT
    engine), hdd = hT * dis (DVE, self rhs for the next layer), stage-A
    for the next table fused here (matmul, *dis, PE-transpose, copy, DMA).
  - mu and logstd heads share one propagation via [Wmu|Wls] concat; the
    output is written feature-major and transposed on host.
  - All index preprocessing host-side; bf16 storage/matmul, fp32 PSUM.
"""

import numpy as np
import ml_dtypes

import concourse.mybir as mybir
import concourse.tile as tile
from concourse import bacc
from concourse import library_config
from concourse.bass_utils import run_bass_kernel_spmd

P = 128
NCORE = 8
N = 50000
NOWN = N // NCORE            # 6250 nodes per core
NB = (NOWN + P - 1) // P     # 49 blocks
NPAD = NB * P                # 6272
VROWS = NCORE * NPAD         # 50176 table rows
VHALF = VROWS // 2           # 25088 (< 2^15, int16-addressable)
KH = 10                      # edge tiles per block per table half
KT = 2 * KH                  # 20 tiles per block total
# Bulk gathers need single_packet=False: with the default True, every
# descriptor must fit one packet per SDMA engine (<=64), and >1024 indices
# hard-crashes the device (NRT_EXEC_UNIT_UNRECOVERABLE).
# GB=1 (one gather per block-half) so real edges form a prefix of each idx
# panel: trailing -1 indices + a runtime count register make the Q7 skip all
# ghost-pad descriptors (~20% of rows).
HB = 25                      # blocks in the first AllGather half
HROWS = HB * P               # 3200 rows
GB = 1
NG = (NB + GB - 1) // GB     # 49 gather groups
GSLOT = GB * KH * P          # idx slots per gather (1280)
NQ = 4                       # SWDGE queues: desc-gen contexts run in parallel

_bf = mybir.dt.bfloat16
_f32 = mybir.dt.float32
_i16 = mybir.dt.int16
_fp8 = mybir.dt.float8e4
_i32 = mybir.dt.int32
_bf_np = ml_dtypes.bfloat16
_fp8_np = ml_dtypes.float8_e4m3

TRACE = False        # set by test harness for profiling runs
TRACE_DIR = None

_cache = {}


def _build_program(use_bias: bool):
    nc = bacc.Bacc("TRN2", num_devices=NCORE, debug=False, num_swdge_queues=NQ)

    Wc = nc.dram_tensor("Wc", [P, 3 * P], _bf, kind="ExternalInput")
    disr = nc.dram_tensor("disr", [P, NPAD], _f32, kind="ExternalInput")
    xds = nc.dram_tensor("xds", [P, NPAD], _bf, kind="ExternalInput")
    ident = nc.dram_tensor("ident", [P, P], _bf, kind="ExternalInput")
    xp = nc.dram_tensor("xp", [P, NB * KT * P], _bf, kind="ExternalInput")
    sS = nc.dram_tensor("sS", [NB * P, KT * P], _fp8, kind="ExternalInput")
    # wrapped int16 gather indices, one [P, GSLOT//16] panel per group per half
    idxAB = nc.dram_tensor("idxAB", [P, 2 * NG * (GSLOT // 16)], _i16,
                           kind="ExternalInput")
    ncnt = nc.dram_tensor("ncnt", [1, 2 * NG], _i32, kind="ExternalInput")
    if use_bias:
        brep = nc.dram_tensor("brep", [P, 3], _f32, kind="ExternalInput")
    outf = nc.dram_tensor("outf", [P, NPAD], _f32, kind="ExternalOutput")
    shard = nc.dram_tensor("shard", [NPAD, P], _bf)
    table = nc.dram_tensor("table", [VROWS, P], _bf, addr_space="Shared")
    wrmin = nc.dram_tensor("wrmin", [1, 16], _bf)
    wrmout = nc.dram_tensor("wrmout", [NCORE, 16], _bf, addr_space="Shared")

    with tile.TileContext(nc) as tc:
        with tc.tile_pool(name="meta", bufs=1) as meta, \
             tc.tile_pool(name="sb", bufs=4) as sb, \
             tc.tile_pool(name="xpp", bufs=2) as xpp, \
             tc.tile_pool(name="mg", bufs=8) as mg, \
             tc.tile_pool(name="ps", bufs=2, space="PSUM") as ps:
            nc.gpsimd.load_library(library_config.mlp)
            # tiny dummy AllGather: warms up the collective rings so the
            # first real AllGather doesn't pay ~85us of setup
            nc.gpsimd.collective_compute(
                "AllGather", mybir.AluOpType.bypass,
                replica_groups=[list(range(NCORE))],
                ins=[wrmin[:]], outs=[wrmout[:]])
            Wc_s = meta.tile([P, 3 * P], _bf)
            nc.sync.dma_start(Wc_s[:], Wc[:])
            ident_s = meta.tile([P, P], _bf)
            nc.sync.dma_start(ident_s[:], ident[:])
            idx_s = meta.tile([P, 2 * NG * (GSLOT // 16)], _i16)
            nc.sync.dma_start(idx_s[:], idxAB[:])
            cnt_s = meta.tile([1, 2 * NG], _i32)
            nc.sync.dma_start(cnt_s[:], ncnt[:])
            disr_s = meta.tile([P, NPAD], _f32)
            nc.sync.dma_start(disr_s[:], disr[:])
            xds_s = meta.tile([P, NPAD], _bf)
            nc.sync.dma_start(xds_s[:], xds[:])
            if use_bias:
                br_s = meta.tile([P, 3], _f32)
                nc.sync.dma_start(br_s[:], brep[:])

            hT_s = meta.tile([P, NPAD], _bf)    # activations, [feat, node]
            hdd_s = meta.tile([P, NPAD], _bf)   # activations * dis^2

            def epilogue(l, b, pb):
                """pb: PSUM [feat, dst-node] pre-norm aggregate for block b."""
                bs = slice(b * P, (b + 1) * P)
                if l < 2:
                    zd = sb.tile([P, P], _f32, tag="zd")
                    nc.vector.tensor_tensor(
                        out=zd[:], in0=pb[:], in1=disr_s[:, bs],
                        op=mybir.AluOpType.mult)
                    if use_bias:
                        nc.vector.tensor_scalar_add(
                            zd[:], zd[:], br_s[:, l:l + 1])
                    nc.scalar.activation(
                        hT_s[:, bs], zd[:],
                        mybir.ActivationFunctionType.Relu)
                    nc.vector.tensor_tensor(
                        out=hdd_s[:, bs], in0=hT_s[:, bs],
                        in1=disr_s[:, bs], op=mybir.AluOpType.mult)
                    # stage A: next layer's table rows for this block
                    py = ps.tile([P, P], _f32, tag="py")
                    nc.tensor.matmul(py[:], lhsT=Wc_s[:, (l + 1) * P:(l + 2) * P],
                                     rhs=hT_s[:, bs], start=True, stop=True)
                    yh = sb.tile([P, P], _bf, tag="yh")
                    nc.vector.tensor_tensor(
                        out=yh[:], in0=py[:], in1=disr_s[:, bs],
                        op=mybir.AluOpType.mult)
                    pt = ps.tile([P, P], _bf, tag="pt")
                    nc.tensor.transpose(pt[:], yh[:], ident_s[:])
                    sh = sb.tile([P, P], _bf, tag="sh")
                    nc.scalar.copy(sh[:], pt[:])
                    nc.sync.dma_start(shard[bs, :], sh[:])
                else:
                    of = sb.tile([P, P], _f32, tag="of")
                    nc.vector.tensor_tensor(
                        out=of[:], in0=pb[:], in1=disr_s[:, bs],
                        op=mybir.AluOpType.mult)
                    if use_bias:
                        nc.vector.tensor_scalar_add(
                            of[:], of[:], br_s[:, 2:3])
                    nc.sync.dma_start(outf[:, bs], of[:])

            def all_gather():
                nc.gpsimd.collective_compute(
                    "AllGather", mybir.AluOpType.bypass,
                    replica_groups=[list(range(NCORE))],
                    ins=[shard[:]], outs=[table[:]])

            # ---- layer 1: no gather; host-permuted Xp tiles as weights.
            # Aggregate raw x*dis rows (incl. self via identity matmul) in
            # input space, then apply W1 once per block: the aggregation is
            # linear, so agg(xW) = agg(x) @ W. Xp streams in 4-block chunks
            # (one contiguous descriptor per partition per chunk).
            CK = 4
            for b in range(NB):
                S = sb.tile([P, KT * P], _fp8, tag="S")
                nc.sync.dma_start(S[:], sS[b * P:(b + 1) * P, :])
                if b % CK == 0:
                    nck = min(CK, NB - b)
                    Xp = xpp.tile([P, CK * KT, P], _bf, tag="Xp")
                    nc.sync.dma_start(
                        Xp[:, 0:nck * KT, :],
                        xp[:, b * KT * P:(b + nck) * KT * P]
                        .rearrange("p (t f) -> p t f", t=nck * KT))
                px = ps.tile([P, P], _f32, tag="px")
                for t in range(KT):
                    nc.tensor.matmul(px[:], lhsT=Xp[:, (b % CK) * KT + t, :],
                                     rhs=S[:, t * P:(t + 1) * P],
                                     start=(t == 0), stop=False)
                nc.tensor.matmul(px[:], lhsT=ident_s[:],
                                 rhs=xds_s[:, b * P:(b + 1) * P],
                                 start=False, stop=True)
                ax = sb.tile([P, P], _bf, tag="ax")
                nc.scalar.copy(ax[:], px[:])
                pb = ps.tile([P, P], _f32, tag="pb")
                nc.tensor.matmul(pb[:], lhsT=Wc_s[:, 0:P], rhs=ax[:],
                                 start=True, stop=True)
                epilogue(0, b, pb)

            all_gather()


            # ---- layers 2/3: gather-based aggregation ----
            rc_cm = nc.gpsimd.register("gcnt")
            rc = rc_cm.__enter__()
            for l in range(1, 3):
                for g in range(NG):
                    b = g
                    MA = mg.tile([P, KH, P], _bf, tag="MA")
                    MB = mg.tile([P, KH, P], _bf, tag="MB")
                    if l == 1 and g < 8:
                        # first touch of each pool slot: clear stale SBUF so
                        # count-skipped tail slots hold 0, not garbage (0*Inf
                        # would poison the S-masked matmul)
                        nc.vector.memset(MA[:], 0)
                        nc.vector.memset(MB[:], 0)
                    cA = (2 * g) * (GSLOT // 16)
                    cB = (2 * g + 1) * (GSLOT // 16)
                    nc.gpsimd.reg_load(rc, cnt_s[0:1, 2 * g:2 * g + 1])
                    nc.gpsimd.dma_gather(
                        MA[:], table[0:VHALF, :],
                        idx_s[:, cA:cA + GSLOT // 16], GSLOT, rc, P,
                        single_packet=False, queue_num=(2 * g) % NQ)
                    nc.gpsimd.reg_load(rc, cnt_s[0:1, 2 * g + 1:2 * g + 2])
                    nc.gpsimd.dma_gather(
                        MB[:], table[VHALF:VROWS, :],
                        idx_s[:, cB:cB + GSLOT // 16], GSLOT, rc, P,
                        single_packet=False, queue_num=(2 * g + 1) % NQ)
                    S = sb.tile([P, KT * P], _fp8, tag="S")
                    nc.sync.dma_start(S[:], sS[b * P:(b + 1) * P, :])
                    pb = ps.tile([P, P], _f32, tag="pb")
                    for t in range(KT):
                        Msrc = MA if t < KH else MB
                        nc.tensor.matmul(pb[:], lhsT=Msrc[:, t % KH, :],
                                         rhs=S[:, t * P:(t + 1) * P],
                                         start=(t == 0), stop=False)
                    nc.tensor.matmul(pb[:], lhsT=Wc_s[:, l * P:(l + 1) * P],
                                     rhs=hdd_s[:, b * P:(b + 1) * P],
                                     start=False, stop=True)
                    epilogue(l, b, pb)
                if l == 1:
                    all_gather()
    nc.compile()
    return nc


def _wrap_idx(idx_flat):
    """dma_gather wrapped layout: slot j at [j%16, j//16], replicated over the
    8 groups of 16 partitions."""
    w = idx_flat.reshape(-1, 16).T          # [16, slots//16]
    return np.tile(w, (8, 1)).astype(np.int16)


def _preprocess(x, edge_index, W1, b1, W2, b2, Wmu, bmu, Wls, bls):
    src_g = np.asarray(edge_index[0]).astype(np.int64)
    dst_g = np.asarray(edge_index[1]).astype(np.int64)
    x = np.asarray(x, dtype=np.float32)

    deg = (np.bincount(dst_g, minlength=N) + 1).astype(np.float32)
    dis = (1.0 / np.sqrt(deg)).astype(np.float32)

    src_core = src_g // NOWN
    tabrow = (src_core * NPAD + (src_g - src_core * NOWN)).astype(np.int64)
    dst_core = dst_g // NOWN

    Wmh = np.concatenate([np.asarray(Wmu), np.asarray(Wls)], axis=1)
    Wc_np = np.concatenate(
        [np.asarray(W1), np.asarray(W2), Wmh], axis=1).astype(_bf_np)
    bmh = np.concatenate([np.asarray(bmu), np.asarray(bls)])
    ball = np.stack([np.asarray(b1), np.asarray(b2), bmh], axis=1)
    use_bias = bool(np.any(ball != 0.0))
    brep_np = ball.astype(np.float32)       # [P, 3]

    ident_np = np.eye(P, dtype=np.float32).astype(_bf_np)
    xsrc = x[src_g] * dis[src_g][:, None]   # per-edge premultiplied source

    in_maps = []
    for c in range(NCORE):
        sel = dst_core == c
        dl = dst_g[sel] - c * NOWN
        tr = tabrow[sel]
        half = (tr >= VHALF).astype(np.int64)
        trh = tr - half * VHALF            # row within half, < 25088
        blocks = dl >> 7
        loc = (dl & 127).astype(np.int64)

        # order by (block, half), then pack each (block, half) bucket into its
        # fixed KH*P slot range
        keys = blocks * 2 + half
        order = np.argsort(keys, kind="stable")
        ksort = keys[order]
        counts = np.bincount(ksort, minlength=2 * NB)
        assert counts.max() <= KH * P, f"block-half overflow: {counts.max()}"
        starts = np.zeros(2 * NB, np.int64)
        starts[1:] = np.cumsum(counts)[:-1]
        pos = np.arange(len(ksort)) - starts[ksort]

        kb = ksort >> 1
        kh = ksort & 1

        # gather idx panels: real edges form a prefix (GB=1), trailing
        # ghosts are -1 and skipped via the runtime count register
        idx_flat = np.full((2 * NG, GSLOT), -1, np.int64)
        idx_flat[2 * kb + kh, pos] = trh[order]
        assert counts.min() >= 1, "empty block-half"
        ncnt_np = counts.astype(np.int32)[None, :]
        idx_panels = np.concatenate(
            [_wrap_idx(idx_flat[i]) for i in range(2 * NG)], axis=1)

        # host-built one-hot S: per block b, S[b*P + slot_part,
        # tile*P + dst_in_block] = 1
        tile_in_b = kh * KH + (pos >> 7)
        prt = pos & 127
        S_np = np.zeros((NB * P, KT * P), np.float32)
        S_np[kb * P + prt, tile_in_b * P + loc[order]] = 1.0

        # host-permuted layer-1 messages: xp[slot_part,
        # (block*KT + tile)*P : +P] = x[src]*dis[src]
        xp3 = np.zeros((P, NB * KT, P), np.float32)
        xp3[prt, kb * KT + tile_in_b, :] = xsrc[sel][order]

        dpad = np.zeros((NPAD,), np.float32)
        dpad[:NOWN] = dis[c * NOWN:(c + 1) * NOWN]
        disr_np = np.tile(dpad[None, :], (P, 1)).astype(np.float32)
        xds_np = (x[c * NOWN:(c + 1) * NOWN]
                  * dpad[:NOWN, None]).T
        xds_full = np.zeros((P, NPAD), np.float32)
        xds_full[:, :NOWN] = xds_np

        im = dict(
            Wc=Wc_np,
            disr=disr_np,
            xds=xds_full.astype(_bf_np),
            ident=ident_np,
            xp=xp3.reshape(P, NB * KT * P).astype(_bf_np),
            sS=S_np.astype(_fp8_np),
            idxAB=idx_panels,
            ncnt=ncnt_np,
        )
        if use_bias:
            im["brep"] = brep_np
        in_maps.append(im)
    return in_maps, use_bias


def kernel(x, edge_index, W1, b1, W2, b2, Wmu, bmu, Wls, bls):
    in_maps, use_bias = _preprocess(
        x, edge_index, W1, b1, W2, b2, Wmu, bmu, Wls, bls)
    if use_bias not in _cache:
        _cache[use_bias] = _build_program(use_bias)
    nc = _cache[use_bias]
    kwargs = {}
    if TRACE:
        kwargs = dict(trace=True, tmpdir=TRACE_DIR)
    res = run_bass_kernel_spmd(nc, in_maps, list(range(NCORE)), **kwargs)
    if TRACE:
        globals()["LAST_RESULT"] = res
    out = np.concatenate(
        [res.results[c]["outf"][:, :NOWN].T for c in range(NCORE)], axis=0)
    mu = np.ascontiguousarray(out[:, :64], dtype=np.float32)
    logstd = np.ascontiguousarray(out[:, 64:], dtype=np.float32)
    return (mu, logstd)
